# revision 15
# baseline (speedup 1.0000x reference)
"""Trainium2 Bass kernel for nn_Block_Order_Aware_Filtering_1_to_1 (v2).

Reference math (B=32, C=128, N=4096, M=512, L=6):
  xs = x[..., 0]                                            [B, C, N]
  Spool = softmax_n(W_pool @ xs)                            [B, M, N]
  h     = einsum('bcn,bmn->bmc', xs, Spool)                 [B, M, C]
  6x:  y = Wf[l] @ h ; BN over (B, C) ; h = relu(yn + h)
  ul  = W_unpool @ xs + b_unpool                            [B, M, N]
  Sun = softmax_m(ul)
  out = einsum('bcm,bmn->bcn', h^T, Sun)[..., None]

Sharding: data-parallel over B across 8 cores (BL=4 batch each); params
replicated; exact global-batch BN parity via one small AllGather of
(sum y, sum y^2) per layer + local 8-way reduction.

v2 layout/scheduling:
  - bf16 operands for all matmuls (PSUM accumulation stays f32); exp has
    no max-subtraction (|logit| <~ 4, exactly equal in exact arithmetic).
  - xsT via DMA xbar transpose (no PE transposes / DVE evacuations).
  - 1/Z row -> all partitions via gpsimd partition_broadcast (PE freed).
  - single long-lived pool set; PSUM budget 4+2+2 = 8 banks so the three
    phases can overlap.
  - unpool logits+exp+Z for batches 0-1 are interleaved into the 6 BN
    AllGather windows; batches 2-3 run inside the output phase.
"""

import os

import numpy as np

import concourse.bass as bass
import concourse.tile as tile
from concourse import bacc, mybir

F32 = mybir.dt.float32
BF16 = mybir.dt.bfloat16
F8 = mybir.dt.float8e4
AF = mybir.ActivationFunctionType
ALU = mybir.AluOpType

B, C, N, M, L = 32, 128, 4096, 512, 6
NCORES = 8
BL = B // NCORES          # 4 batch items per core
NT = N // 128             # 32 n-chunks of 128
MT = M // 128             # 4 m/o-chunks of 128
NJ = N // 512             # 8 n-tiles of 512
EPS = 1e-5
INV_BC = 1.0 / float(B * C)

DBG_LAYERS = int(os.environ.get("BASSK_LAYERS", str(L)))
DBG_NOAR = os.environ.get("BASSK_NOAR", "0") == "1"
DBG_REPS = int(os.environ.get("BASSK_REPS", "1"))

# how many unpool-pre units (batch, nj-pair) to emit inside each filter
# layer's collective window; 16 units total (4 batches x 4 nj-pairs)
UNITS_PER_LAYER = [3, 3, 3, 3, 2, 2] if L == 6 else [16] + [0] * (L - 1)


def _kernel_body(nc, tc, x_d, wpT_d, wfT_d, gamma_d, beta_d, wuT_d, bu_d,
                 ident_d, ones_d, out_d):
    with (
        tc.tile_pool(name="const", bufs=1) as constp,
        tc.tile_pool(name="xs", bufs=BL) as xsp,
        tc.tile_pool(name="xsT", bufs=1) as xsTp,
        tc.tile_pool(name="E", bufs=4) as Ep,
        tc.tile_pool(name="h", bufs=8) as hp,
        tc.tile_pool(name="wf", bufs=8) as wfp,
        tc.tile_pool(name="ysb", bufs=4) as ysbp,
        tc.tile_pool(name="sq", bufs=2) as sqp,
        tc.tile_pool(name="st", bufs=2) as stp,
        tc.tile_pool(name="eu", bufs=16) as eup,
        tc.tile_pool(name="rz", bufs=1) as rzp,
        tc.tile_pool(name="rzb", bufs=3) as rzbp,
        tc.tile_pool(name="osb", bufs=2) as osbp,
        tc.tile_pool(name="ps1", bufs=2, space="PSUM") as ps1,   # 2x[128,1024]
        tc.tile_pool(name="ps2", bufs=2, space="PSUM") as ps2,   # 2x[128,512]
        tc.tile_pool(name="ps3", bufs=2, space="PSUM") as ps3,   # 2x 1 bank
        tc.tile_pool(name="dram", bufs=2, space="DRAM") as dramp,
    ):
        # ---------------- constants ----------------
        ident = constp.tile([128, 128], BF16)
        nc.sync.dma_start(out=ident, in_=ident_d)
        ones_col = constp.tile([128, 1], BF16)
        nc.sync.dma_start(out=ones_col, in_=ones_d[:, 0:1])
        ones8 = constp.tile([128, 1], F8)
        nc.vector.memset(ones8, 1.0)
        wp_sb = constp.tile([C, M], BF16)
        nc.sync.dma_start(out=wp_sb, in_=wpT_d)
        wu_sb = constp.tile([C, M], BF16)
        nc.sync.dma_start(out=wu_sb, in_=wuT_d)
        gb_sb = constp.tile([128, L * MT], F32)
        nc.sync.dma_start(out=gb_sb.rearrange("p (l o) -> p l o", l=L),
                          in_=gamma_d.rearrange("l (o p) -> p l o", p=128))
        bb_sb = constp.tile([128, L * MT], F32)
        nc.sync.dma_start(out=bb_sb.rearrange("p (l o) -> p l o", l=L),
                          in_=beta_d.rearrange("l (o p) -> p l o", p=128))
        eps_sb = constp.tile([128, 1], F32)
        nc.vector.memset(eps_sb, EPS)
        bu_sb = constp.tile([128, MT], F32)
        nc.sync.dma_start(out=bu_sb, in_=bu_d.rearrange("(o p) -> p o", p=128))

        for _rep in range(DBG_REPS):
            # ---------------- pool phase ----------------
            xs_tiles = []
            h0 = [hp.tile([128, BL * C], BF16, name=f"h_0_{mi}", tag="h")
                  for mi in range(MT)]
            for b in range(BL):
                xs_sb = xsp.tile([C, N], BF16, name=f"xs_{b}", tag="xs")
                nc.sync.dma_start(out=xs_sb, in_=x_d[b])
                xs_tiles.append(xs_sb)
                xsT_sb = xsTp.tile([128, NT * C], BF16, name=f"xsT_{b}",
                                   tag="xsT")
                nc.sync.dma_start_transpose(
                    out=xsT_sb.rearrange("p (t c) -> p t c", t=NT),
                    in_=x_d[b])

                ps_hT = ps2.tile([128, M], F32, name=f"hT_{b}", tag="p2")
                ps_z = ps2.tile([1, M], F32, name=f"z_{b}", tag="p2")

                def consume(g, b=b, xsT_sb=xsT_sb, ps_hT=ps_hT, ps_z=ps_z,
                            E_sb=None):
                    for k in range(2):
                        ni = 2 * g + k
                        nc.tensor.matmul(
                            ps_hT, xsT_sb[:, ni * C:(ni + 1) * C],
                            E_sb[ni][:, (ni % 2) * M:(ni % 2 + 1) * M],
                            start=(ni == 0), stop=(ni == NT - 1))
                        nc.tensor.matmul(
                            ps_z[0:1, :], ones_col,
                            E_sb[ni][:, (ni % 2) * M:(ni % 2 + 1) * M],
                            start=(ni == 0), stop=(ni == NT - 1))

                E_of_ni = {}
                for g in range(NT // 2):
                    ps_log = ps1.tile([128, 2 * M], F32, name=f"log_{b}_{g}",
                                      tag="p1")
                    for k in range(2):
                        ni = 2 * g + k
                        nc.tensor.matmul(ps_log[:, k * M:(k + 1) * M],
                                         xs_sb[:, ni * 128:(ni + 1) * 128],
                                         wp_sb, start=True, stop=True)
                    e_t = Ep.tile([128, 2 * M], BF16, name=f"E_{b}_{g}",
                                  tag="E")
                    with nc.allow_low_precision("softmax weights in bf16"):
                        nc.scalar.activation(out=e_t, in_=ps_log, func=AF.Exp)
                    E_of_ni[2 * g] = e_t
                    E_of_ni[2 * g + 1] = e_t
                    if g >= 1:
                        consume(g - 1, E_sb=E_of_ni)
                consume(NT // 2 - 1, E_sb=E_of_ni)

                rz = rzp.tile([1, M], F32, name=f"rz_{b}", tag="rzpool",
                              bufs=2)
                nc.vector.reciprocal(out=rz, in_=ps_z[0:1, :])
                rzb = rzbp.tile([128, M], F32, name=f"rzb_{b}", tag="rzbp")
                nc.gpsimd.partition_broadcast(rzb, rz)
                hTs = rzp.tile([128, M], BF16, name=f"hTs_{b}", tag="hTs",
                               bufs=2)
                with nc.allow_low_precision("pooled h in bf16"):
                    nc.vector.tensor_mul(hTs, ps_hT, rzb)
                for mi in range(MT):
                    ps_h = ps2.tile([128, 128], BF16, name=f"h_{b}_{mi}",
                                    tag="p2")
                    nc.tensor.transpose(ps_h, hTs[:, mi * 128:(mi + 1) * 128],
                                        ident)
                    nc.vector.tensor_copy(out=h0[mi][:, b * C:(b + 1) * C],
                                          in_=ps_h)

            # -------- filter phase with interleaved unpool-pre slabs -------
            eu_tiles = {}    # (b, mi) -> [128, N] fp8e4m3
            rz_all = {}      # b -> [1, NJ*512] bf16
            slab_units = [(b, j2) for b in range(BL) for j2 in range(NJ // 2)]
            unit_pos = 0

            def emit_slab_unit(b, j2):
                xs_sb = xs_tiles[b]
                if b not in rz_all:
                    rz_all[b] = rzp.tile([1, NJ * 512], BF16,
                                         name=f"rzu_{b}", tag=f"rzu{b}",
                                         bufs=1)
                eus = []
                for mi in range(MT):
                    if (b, mi) not in eu_tiles:
                        eu_tiles[(b, mi)] = eup.tile(
                            [128, N], F8, name=f"eu_{b}_{mi}", tag="eu")
                    eu = eu_tiles[(b, mi)]
                    ps_ul = ps1.tile([128, 1024], F32,
                                     name=f"ul_{b}_{j2}_{mi}", tag="p1")
                    for k in range(2):
                        nj = 2 * j2 + k
                        nc.tensor.matmul(
                            ps_ul[:, k * 512:(k + 1) * 512],
                            wu_sb[:, mi * 128:(mi + 1) * 128],
                            xs_sb[:, nj * 512:(nj + 1) * 512],
                            start=True, stop=True)
                    with nc.allow_low_precision("softmax weights in bf16"):
                        nc.scalar.activation(
                            out=eu[:, 2 * j2 * 512:(2 * j2 + 2) * 512],
                            in_=ps_ul, func=AF.Exp,
                            bias=bu_sb[:, mi:mi + 1])
                    eus.append(eu)
                for k in range(2):
                    nj = 2 * j2 + k
                    ps_zu = ps3.tile([1, 512], F32, name=f"zu_{b}_{nj}",
                                     tag="p3z")
                    for mi in range(MT):
                        nc.tensor.matmul(
                            ps_zu[0:1, :], ones8,
                            eus[mi][:, nj * 512:(nj + 1) * 512],
                            start=(mi == 0), stop=(mi == MT - 1))
                    with nc.allow_low_precision("1/Z in bf16"):
                        nc.vector.reciprocal(
                            out=rz_all[b][:, nj * 512:(nj + 1) * 512],
                            in_=ps_zu[0:1, :])

            h_cur = h0
            wf_sb = {}

            def load_wf(l):
                if l >= DBG_LAYERS:
                    return
                tiles = []
                for mi in range(MT):
                    w = wfp.tile([128, M], BF16, name=f"wf_{l}_{mi}",
                                 tag="wf")
                    nc.gpsimd.dma_start(
                        out=w, in_=wfT_d[l, mi * 128:(mi + 1) * 128, :])
                    tiles.append(w)
                wf_sb[l] = tiles

            load_wf(0)
            load_wf(1)

            for l in range(DBG_LAYERS):
                stats = stp.tile([128, 2 * MT], F32, name=f"st_{l}",
                                 tag="st")
                y_sb = []
                for oi in range(MT):
                    ps_y = ps2.tile([128, BL * C], F32, name=f"y_{l}_{oi}",
                                    tag="p2")
                    for mi in range(MT):
                        nc.tensor.matmul(
                            ps_y, wf_sb[l][mi][:, oi * 128:(oi + 1) * 128],
                            h_cur[mi], start=(mi == 0), stop=(mi == MT - 1))
                    y = ysbp.tile([128, BL * C], F32, name=f"ysb_{l}_{oi}",
                                  tag="ysb")
                    nc.scalar.activation(out=y, in_=ps_y, func=AF.Copy,
                                         accum_out=stats[:, oi:oi + 1])
                    sq = sqp.tile([128, BL * C], F32, name=f"sq_{l}_{oi}",
                                  tag="sq")
                    nc.vector.scalar_tensor_tensor(
                        out=sq, in0=y, scalar=1.0, in1=y,
                        op0=ALU.mult, op1=ALU.mult,
                        accum_out=stats[:, MT + oi:MT + oi + 1])
                    y_sb.append(y)

                # exchange partial sums -> exact global-batch BN stats
                gsum = stp.tile([128, 2 * MT], F32, name=f"gsum_{l}",
                                tag="gsum")
                if DBG_NOAR:
                    nc.vector.tensor_scalar_mul(gsum, stats, float(NCORES))
                else:
                    st_in = dramp.tile([128, 2 * MT], F32, name=f"sti_{l}",
                                       tag=f"sti{l}", bufs=1)
                    ag_out = dramp.tile([NCORES * 128, 2 * MT], F32,
                                        name=f"sto_{l}", tag=f"sto{l}",
                                        bufs=1, addr_space="Shared")
                    nc.sync.dma_start(out=st_in, in_=stats)
                    nc.gpsimd.collective_compute(
                        "AllGather", ALU.bypass,
                        replica_groups=[list(range(NCORES))],
                        ins=[st_in.opt()], outs=[ag_out.opt()])
                    gst = stp.tile([128, NCORES * 2 * MT], F32,
                                   name=f"gst_{l}", tag="gst")
                    gst_v = gst.rearrange("p (s t) -> p s t", s=NCORES)
                    ag_v = ag_out.rearrange("(s p) t -> p s t", s=NCORES)
                    half = NCORES // 2
                    nc.sync.dma_start(out=gst_v[:, 0:half],
                                      in_=ag_v[:, 0:half])
                    nc.gpsimd.dma_start(out=gst_v[:, half:NCORES],
                                        in_=ag_v[:, half:NCORES])

                # fill the collective window with unpool pre-work
                for _ in range(UNITS_PER_LAYER[l]):
                    if unit_pos < len(slab_units):
                        emit_slab_unit(*slab_units[unit_pos])
                        unit_pos += 1
                load_wf(l + 2)

                if not DBG_NOAR:
                    t1 = stp.tile([128, NCORES * MT], F32, name=f"t1_{l}",
                                  tag="t1")
                    nc.vector.tensor_add(t1, gst[:, 0:NCORES * MT],
                                         gst[:, NCORES * MT:2 * NCORES * MT])
                    t2 = stp.tile([128, NCORES * 2], F32, name=f"t2_{l}",
                                  tag="t2")
                    nc.vector.tensor_add(t2, t1[:, 0:NCORES * 2],
                                         t1[:, NCORES * 2:NCORES * MT])
                    nc.vector.tensor_add(gsum, t2[:, 0:2 * MT],
                                         t2[:, 2 * MT:NCORES * 2])

                mean = stp.tile([128, MT], F32, name=f"mean_{l}", tag="mean")
                nc.vector.tensor_scalar_mul(mean, gsum[:, 0:MT], INV_BC)
                msq = stp.tile([128, MT], F32, name=f"msq_{l}", tag="msq")
                nc.vector.tensor_scalar_mul(msq, gsum[:, MT:2 * MT], INV_BC)
                m2 = stp.tile([128, MT], F32, name=f"m2_{l}", tag="m2")
                nc.vector.tensor_mul(m2, mean, mean)
                var = stp.tile([128, MT], F32, name=f"var_{l}", tag="var")
                nc.vector.scalar_tensor_tensor(
                    out=var, in0=m2, scalar=-1.0, in1=msq,
                    op0=ALU.mult, op1=ALU.add)
                std = stp.tile([128, MT], F32, name=f"std_{l}", tag="std")
                nc.scalar.activation(out=std, in_=var, func=AF.Sqrt,
                                     bias=eps_sb)
                rstd = stp.tile([128, MT], F32, name=f"rstd_{l}", tag="rstd")
                nc.vector.reciprocal(out=rstd, in_=std)
                a_t = stp.tile([128, MT], F32, name=f"a_{l}", tag="a")
                nc.vector.tensor_mul(a_t, gb_sb[:, l * MT:(l + 1) * MT], rstd)
                ma = stp.tile([128, MT], F32, name=f"ma_{l}", tag="ma")
                nc.vector.tensor_mul(ma, mean, a_t)
                b_t = stp.tile([128, MT], F32, name=f"b_{l}", tag="b")
                nc.vector.scalar_tensor_tensor(
                    out=b_t, in0=ma, scalar=-1.0,
                    in1=bb_sb[:, l * MT:(l + 1) * MT],
                    op0=ALU.mult, op1=ALU.add)

                h_next = []
                for oi in range(MT):
                    tmp = sqp.tile([128, BL * C], F32, name=f"tmp_{l}_{oi}",
                                   tag="sq")
                    nc.vector.scalar_tensor_tensor(
                        out=tmp, in0=y_sb[oi], scalar=a_t[:, oi:oi + 1],
                        in1=h_cur[oi], op0=ALU.mult, op1=ALU.add)
                    hn = hp.tile([128, BL * C], BF16, name=f"h_{l + 1}_{oi}",
                                 tag="h")
                    with nc.allow_low_precision("h state in bf16"):
                        nc.scalar.activation(out=hn, in_=tmp, func=AF.Relu,
                                             bias=b_t[:, oi:oi + 1])
                    h_next.append(hn)
                h_cur = h_next
            h_fin = h_cur

            # ---------------- output phase ----------------
            h_fin8 = []
            for mi in range(MT):
                h8 = rzp.tile([128, BL * C], F8, name=f"h8_{mi}", tag="h8",
                              bufs=MT)
                with nc.allow_low_precision("h_fin in fp8 for output matmul"):
                    nc.vector.tensor_copy(out=h8, in_=h_fin[mi])
                h_fin8.append(h8)

            def emit_out_batch(b):
                dma_engines = [nc.sync, nc.scalar, nc.gpsimd, nc.sync]
                for j2 in range(NJ // 2):
                    ps_o = ps1.tile([128, 1024], F32, name=f"o_{b}_{j2}",
                                    tag="p1")
                    for mi in range(MT):
                        for k in range(2):
                            nj = 2 * j2 + k
                            nc.tensor.matmul(
                                ps_o[:, k * 512:(k + 1) * 512],
                                h_fin8[mi][:, b * C:(b + 1) * C],
                                eu_tiles[(b, mi)][:, nj * 512:(nj + 1) * 512],
                                start=(mi == 0), stop=(mi == MT - 1))
                    rzb = rzbp.tile([128, 1024], BF16, name=f"rzbo_{b}_{j2}",
                                    tag="rzbo")
                    nc.gpsimd.partition_broadcast(
                        rzb, rz_all[b][:, 2 * j2 * 512:(2 * j2 + 2) * 512])
                    o_sb = osbp.tile([128, 1024], F32, name=f"os_{b}_{j2}",
                                     tag="os")
                    nc.vector.tensor_mul(o_sb, ps_o, rzb)
                    dma_engines[j2].dma_start(
                        out=out_d[b, :, 2 * j2 * 512:(2 * j2 + 2) * 512],
                        in_=o_sb)

            while unit_pos < len(slab_units):
                emit_slab_unit(*slab_units[unit_pos])
                unit_pos += 1
            for b in range(BL):
                emit_out_batch(b)


_CACHE = {}


def build():
    if "nc" in _CACHE:
        return _CACHE["nc"]
    nc = bacc.Bacc("TRN2", target_bir_lowering=False, debug=False,
                   num_devices=NCORES)
    x_d = nc.dram_tensor("x", [BL, C, N], BF16, kind="ExternalInput").ap()
    wpT_d = nc.dram_tensor("w_pool_t", [C, M], BF16, kind="ExternalInput").ap()
    wfT_d = nc.dram_tensor("wf_t", [L, M, M], BF16, kind="ExternalInput").ap()
    gamma_d = nc.dram_tensor("gamma", [L, M], F32, kind="ExternalInput").ap()
    beta_d = nc.dram_tensor("beta", [L, M], F32, kind="ExternalInput").ap()
    wuT_d = nc.dram_tensor("w_unpool_t", [C, M], BF16,
                           kind="ExternalInput").ap()
    bu_d = nc.dram_tensor("b_unpool", [M], F32, kind="ExternalInput").ap()
    ident_d = nc.dram_tensor("ident", [128, 128], BF16,
                             kind="ExternalInput").ap()
    ones_d = nc.dram_tensor("ones", [128, 128], BF16,
                            kind="ExternalInput").ap()
    out_d = nc.dram_tensor("out", [BL, C, N], F32, kind="ExternalOutput").ap()

    with tile.TileContext(nc) as tc:
        _kernel_body(nc, tc, x_d, wpT_d, wfT_d, gamma_d, beta_d, wuT_d, bu_d,
                     ident_d, ones_d, out_d)
    nc.compile()
    _CACHE["nc"] = nc
    return nc


def make_in_maps(x, W_pool, Wf, gamma, beta, W_unpool, b_unpool):
    bf16 = mybir.dt.np(BF16)
    xs = np.ascontiguousarray(
        np.asarray(x, dtype=np.float32)[..., 0]).astype(bf16)
    shards = xs.reshape(NCORES, BL, C, N)
    wpT = np.ascontiguousarray(np.asarray(W_pool, np.float32).T).astype(bf16)
    wfT = np.ascontiguousarray(
        np.asarray(Wf, np.float32).transpose(0, 2, 1)).astype(bf16)
    wuT = np.ascontiguousarray(np.asarray(W_unpool, np.float32).T).astype(bf16)
    common = {
        "w_pool_t": wpT, "wf_t": wfT,
        "gamma": np.ascontiguousarray(np.asarray(gamma, np.float32)),
        "beta": np.ascontiguousarray(np.asarray(beta, np.float32)),
        "w_unpool_t": wuT,
        "b_unpool": np.ascontiguousarray(np.asarray(b_unpool, np.float32)),
        "ident": np.eye(128, dtype=np.float32).astype(bf16),
        "ones": np.ones((128, 128), dtype=np.float32).astype(bf16),
    }
    return [{"x": np.ascontiguousarray(shards[i]), **common}
            for i in range(NCORES)]


LAST_RESULTS = None


def kernel(x, W_pool, Wf, gamma, beta, W_unpool, b_unpool, trace=False):
    global LAST_RESULTS
    from concourse.bass_utils import run_bass_kernel_spmd
    nc = build()
    in_maps = make_in_maps(x, W_pool, Wf, gamma, beta, W_unpool, b_unpool)
    res = run_bass_kernel_spmd(nc, in_maps, core_ids=list(range(NCORES)),
                               trace=trace)
    LAST_RESULTS = res
    out = np.concatenate([res.results[i]["out"] for i in range(NCORES)],
                         axis=0)
    return out.reshape(B, C, N, 1)


# revision 22
# speedup vs baseline: 1.9906x; 1.9906x over previous
"""Trainium2 Bass kernel for nn_Block_Order_Aware_Filtering_1_to_1 (v2).

Reference math (B=32, C=128, N=4096, M=512, L=6):
  xs = x[..., 0]                                            [B, C, N]
  Spool = softmax_n(W_pool @ xs)                            [B, M, N]
  h     = einsum('bcn,bmn->bmc', xs, Spool)                 [B, M, C]
  6x:  y = Wf[l] @ h ; BN over (B, C) ; h = relu(yn + h)
  ul  = W_unpool @ xs + b_unpool                            [B, M, N]
  Sun = softmax_m(ul)
  out = einsum('bcm,bmn->bcn', h^T, Sun)[..., None]

Sharding: data-parallel over B across 8 cores (BL=4 batch each); params
replicated; exact global-batch BN parity via one small AllGather of
(sum y, sum y^2) per layer + local 8-way reduction.

v2 layout/scheduling:
  - bf16 operands for all matmuls (PSUM accumulation stays f32); exp has
    no max-subtraction (|logit| <~ 4, exactly equal in exact arithmetic).
  - xsT via DMA xbar transpose (no PE transposes / DVE evacuations).
  - 1/Z row -> all partitions via gpsimd partition_broadcast (PE freed).
  - single long-lived pool set; PSUM budget 4+2+2 = 8 banks so the three
    phases can overlap.
  - unpool logits+exp+Z for batches 0-1 are interleaved into the 6 BN
    AllGather windows; batches 2-3 run inside the output phase.
"""

import os

import numpy as np

import concourse.bass as bass
import concourse.tile as tile
from concourse import bacc, mybir

F32 = mybir.dt.float32
BF16 = mybir.dt.bfloat16
F8 = mybir.dt.float8e4
AF = mybir.ActivationFunctionType
ALU = mybir.AluOpType

B, C, N, M, L = 32, 128, 4096, 512, 6
NCORES = 8
BL = B // NCORES          # 4 batch items per core
NT = N // 128             # 32 n-chunks of 128
MT = M // 128             # 4 m/o-chunks of 128
NJ = N // 512             # 8 n-tiles of 512
EPS = 1e-5
INV_BC = 1.0 / float(B * C)

DBG_LAYERS = int(os.environ.get("BASSK_LAYERS", str(L)))
DBG_NOAR = os.environ.get("BASSK_NOAR", "0") == "1"
DBG_REPS = int(os.environ.get("BASSK_REPS", "1"))
DBG_AR = os.environ.get("BASSK_AR", "0") == "1"        # AllReduce not AllGather
DBG_NOPB = os.environ.get("BASSK_NOPB", "0") == "1"    # PE bcast, no gpsimd pb
DBG_NODMAT = os.environ.get("BASSK_NODMAT", "0") == "1"  # PE transposes

# how many unpool-pre units (batch, nj-pair) to emit inside each filter
# layer's collective window; 16 units total (4 batches x 4 nj-pairs)
UNITS_PER_LAYER = [3, 3, 3, 3, 2, 2] if L == 6 else [16] + [0] * (L - 1)


def _kernel_body(nc, tc, x_d, wpT_d, wfT_d, gamma_d, beta_d, wuT_d, bu_d,
                 ident_d, ones_d, out_d):
    with (
        tc.tile_pool(name="const", bufs=1) as constp,
        tc.tile_pool(name="xs", bufs=BL) as xsp,
        tc.tile_pool(name="xsT", bufs=1) as xsTp,
        tc.tile_pool(name="E", bufs=4) as Ep,
        tc.tile_pool(name="h", bufs=8) as hp,
        tc.tile_pool(name="wf", bufs=8) as wfp,
        tc.tile_pool(name="ysb", bufs=4) as ysbp,
        tc.tile_pool(name="sq", bufs=2) as sqp,
        tc.tile_pool(name="st", bufs=2) as stp,
        tc.tile_pool(name="eu", bufs=16) as eup,
        tc.tile_pool(name="rz", bufs=1) as rzp,
        tc.tile_pool(name="rzb", bufs=3) as rzbp,
        tc.tile_pool(name="osb", bufs=2) as osbp,
        tc.tile_pool(name="ps1", bufs=2, space="PSUM") as ps1,   # 2x[128,1024]
        tc.tile_pool(name="ps2", bufs=2, space="PSUM") as ps2,   # 2x[128,512]
        tc.tile_pool(name="ps3", bufs=2, space="PSUM") as ps3,   # 2x 1 bank
        tc.tile_pool(name="dram", bufs=2, space="DRAM") as dramp,
    ):
        # ---------------- constants ----------------
        ident = constp.tile([128, 128], BF16)
        nc.sync.dma_start(out=ident, in_=ident_d)
        ones_col = constp.tile([128, 1], BF16)
        nc.sync.dma_start(out=ones_col, in_=ones_d[:, 0:1])
        ones8 = constp.tile([128, 1], F8)
        nc.vector.memset(ones8, 1.0)
        ones_row = constp.tile([1, 128], BF16)
        nc.sync.dma_start(out=ones_row, in_=ones_d[0:1, :])

        def bcast(out_sb, row_ap, psp, tag):
            """Broadcast [1, W] row_ap to all 128 partitions of out_sb."""
            if DBG_NOPB:
                w = row_ap.free_size()
                ps_rb = psp.tile([128, w], F32, tag=tag)
                nc.tensor.matmul(ps_rb, ones_row, row_ap, start=True,
                                 stop=True)
                nc.vector.tensor_copy(out=out_sb, in_=ps_rb)
            else:
                nc.gpsimd.partition_broadcast(out_sb, row_ap)
        wp_sb = constp.tile([C, M], BF16)
        nc.sync.dma_start(out=wp_sb, in_=wpT_d)
        wu_sb = constp.tile([C, M], BF16)
        nc.sync.dma_start(out=wu_sb, in_=wuT_d)
        gb_sb = constp.tile([128, L * MT], F32)
        nc.sync.dma_start(out=gb_sb.rearrange("p (l o) -> p l o", l=L),
                          in_=gamma_d.rearrange("l (o p) -> p l o", p=128))
        bb_sb = constp.tile([128, L * MT], F32)
        nc.sync.dma_start(out=bb_sb.rearrange("p (l o) -> p l o", l=L),
                          in_=beta_d.rearrange("l (o p) -> p l o", p=128))
        eps_sb = constp.tile([128, 1], F32)
        nc.vector.memset(eps_sb, EPS)
        bu_sb = constp.tile([128, MT], F32)
        nc.sync.dma_start(out=bu_sb, in_=bu_d.rearrange("(o p) -> p o", p=128))

        for _rep in range(DBG_REPS):
            # ---------------- pool phase ----------------
            xs_tiles = []
            h0 = [hp.tile([128, BL * C], BF16, name=f"h_0_{mi}", tag="h")
                  for mi in range(MT)]
            for b in range(BL):
                xs_sb = xsp.tile([C, N], BF16, name=f"xs_{b}", tag="xs")
                nc.sync.dma_start(out=xs_sb, in_=x_d[b])
                xs_tiles.append(xs_sb)
                xsT_sb = xsTp.tile([128, NT * C], BF16, name=f"xsT_{b}",
                                   tag="xsT")
                if DBG_NODMAT:
                    for ni in range(NT):
                        ps_t = ps3.tile([128, 128], BF16,
                                        name=f"xt_{b}_{ni}", tag="p3z")
                        nc.tensor.transpose(
                            ps_t, xs_sb[:, ni * 128:(ni + 1) * 128], ident)
                        nc.vector.tensor_copy(
                            out=xsT_sb[:, ni * C:(ni + 1) * C], in_=ps_t)
                else:
                    nc.sync.dma_start_transpose(
                        out=xsT_sb.rearrange("p (t c) -> p t c", t=NT),
                        in_=x_d[b])

                ps_hT = ps2.tile([128, M], F32, name=f"hT_{b}", tag="p2")
                ps_z = ps2.tile([1, M], F32, name=f"z_{b}", tag="p2")

                def consume(g, b=b, xsT_sb=xsT_sb, ps_hT=ps_hT, ps_z=ps_z,
                            E_sb=None):
                    for k in range(2):
                        ni = 2 * g + k
                        nc.tensor.matmul(
                            ps_hT, xsT_sb[:, ni * C:(ni + 1) * C],
                            E_sb[ni][:, (ni % 2) * M:(ni % 2 + 1) * M],
                            start=(ni == 0), stop=(ni == NT - 1))
                        nc.tensor.matmul(
                            ps_z[0:1, :], ones_col,
                            E_sb[ni][:, (ni % 2) * M:(ni % 2 + 1) * M],
                            start=(ni == 0), stop=(ni == NT - 1))

                E_of_ni = {}
                for g in range(NT // 2):
                    ps_log = ps1.tile([128, 2 * M], F32, name=f"log_{b}_{g}",
                                      tag="p1")
                    for k in range(2):
                        ni = 2 * g + k
                        nc.tensor.matmul(ps_log[:, k * M:(k + 1) * M],
                                         xs_sb[:, ni * 128:(ni + 1) * 128],
                                         wp_sb, start=True, stop=True)
                    e_t = Ep.tile([128, 2 * M], BF16, name=f"E_{b}_{g}",
                                  tag="E")
                    with nc.allow_low_precision("softmax weights in bf16"):
                        nc.scalar.activation(out=e_t, in_=ps_log, func=AF.Exp)
                    E_of_ni[2 * g] = e_t
                    E_of_ni[2 * g + 1] = e_t
                    if g >= 1:
                        consume(g - 1, E_sb=E_of_ni)
                consume(NT // 2 - 1, E_sb=E_of_ni)

                rz = rzp.tile([1, M], BF16, name=f"rz_{b}", tag="rzpool",
                              bufs=2)
                with nc.allow_low_precision("pool 1/Z in bf16"):
                    nc.vector.reciprocal(out=rz, in_=ps_z[0:1, :])
                rzb = rzbp.tile([128, M], BF16, name=f"rzb_{b}", tag="rzbp")
                bcast(rzb, rz, ps3, "p3z")
                hTs = rzp.tile([128, M], BF16, name=f"hTs_{b}", tag="hTs",
                               bufs=2)
                with nc.allow_low_precision("pooled h in bf16"):
                    nc.vector.tensor_mul(hTs, ps_hT, rzb)
                for mi in range(MT):
                    ps_h = ps2.tile([128, 128], BF16, name=f"h_{b}_{mi}",
                                    tag="p2")
                    nc.tensor.transpose(ps_h, hTs[:, mi * 128:(mi + 1) * 128],
                                        ident)
                    nc.vector.tensor_copy(out=h0[mi][:, b * C:(b + 1) * C],
                                          in_=ps_h)

            # -------- filter phase with interleaved unpool-pre slabs -------
            eu_tiles = {}    # (b, mi) -> [128, N] fp8e4m3
            rz_all = {}      # b -> [1, NJ*512] bf16
            slab_units = [(b, j2) for b in range(BL) for j2 in range(NJ // 2)]
            unit_pos = 0

            def emit_slab_unit(b, j2):
                xs_sb = xs_tiles[b]
                if b not in rz_all:
                    rz_all[b] = rzp.tile([1, NJ * 512], BF16,
                                         name=f"rzu_{b}", tag=f"rzu{b}",
                                         bufs=1)
                eus = []
                for mi in range(MT):
                    if (b, mi) not in eu_tiles:
                        eu_tiles[(b, mi)] = eup.tile(
                            [128, N], F8, name=f"eu_{b}_{mi}", tag="eu")
                    eu = eu_tiles[(b, mi)]
                    ps_ul = ps1.tile([128, 1024], F32,
                                     name=f"ul_{b}_{j2}_{mi}", tag="p1")
                    for k in range(2):
                        nj = 2 * j2 + k
                        nc.tensor.matmul(
                            ps_ul[:, k * 512:(k + 1) * 512],
                            wu_sb[:, mi * 128:(mi + 1) * 128],
                            xs_sb[:, nj * 512:(nj + 1) * 512],
                            start=True, stop=True)
                    with nc.allow_low_precision("softmax weights in bf16"):
                        nc.scalar.activation(
                            out=eu[:, 2 * j2 * 512:(2 * j2 + 2) * 512],
                            in_=ps_ul, func=AF.Exp,
                            bias=bu_sb[:, mi:mi + 1])
                    eus.append(eu)
                for k in range(2):
                    nj = 2 * j2 + k
                    ps_zu = ps3.tile([1, 512], F32, name=f"zu_{b}_{nj}",
                                     tag="p3z")
                    for mi in range(MT):
                        nc.tensor.matmul(
                            ps_zu[0:1, :], ones8,
                            eus[mi][:, nj * 512:(nj + 1) * 512],
                            start=(mi == 0), stop=(mi == MT - 1))
                    with nc.allow_low_precision("1/Z in bf16"):
                        nc.vector.reciprocal(
                            out=rz_all[b][:, nj * 512:(nj + 1) * 512],
                            in_=ps_zu[0:1, :])

            h_cur = h0
            wf_sb = {}

            def load_wf(l):
                if l >= DBG_LAYERS:
                    return
                tiles = []
                for mi in range(MT):
                    w = wfp.tile([128, M], BF16, name=f"wf_{l}_{mi}",
                                 tag="wf")
                    nc.gpsimd.dma_start(
                        out=w, in_=wfT_d[l, mi * 128:(mi + 1) * 128, :])
                    tiles.append(w)
                wf_sb[l] = tiles

            load_wf(0)
            load_wf(1)

            for l in range(DBG_LAYERS):
                stats = stp.tile([128, 2 * MT], F32, name=f"st_{l}",
                                 tag="st")
                y_sb = []
                for oi in range(MT):
                    ps_y = ps2.tile([128, BL * C], F32, name=f"y_{l}_{oi}",
                                    tag="p2")
                    for mi in range(MT):
                        nc.tensor.matmul(
                            ps_y, wf_sb[l][mi][:, oi * 128:(oi + 1) * 128],
                            h_cur[mi], start=(mi == 0), stop=(mi == MT - 1))
                    y = ysbp.tile([128, BL * C], F32, name=f"ysb_{l}_{oi}",
                                  tag="ysb")
                    nc.scalar.activation(out=y, in_=ps_y, func=AF.Copy,
                                         accum_out=stats[:, oi:oi + 1])
                    sq = sqp.tile([128, BL * C], F32, name=f"sq_{l}_{oi}",
                                  tag="sq")
                    nc.vector.scalar_tensor_tensor(
                        out=sq, in0=y, scalar=1.0, in1=y,
                        op0=ALU.mult, op1=ALU.mult,
                        accum_out=stats[:, MT + oi:MT + oi + 1])
                    y_sb.append(y)

                # exchange partial sums -> exact global-batch BN stats
                gsum = stp.tile([128, 2 * MT], F32, name=f"gsum_{l}",
                                tag="gsum")
                if DBG_NOAR:
                    nc.vector.tensor_scalar_mul(gsum, stats, float(NCORES))
                elif DBG_AR:
                    st_in = dramp.tile([128, 2 * MT], F32, name=f"sti_{l}",
                                       tag=f"sti{l}", bufs=1)
                    st_out = dramp.tile([128, 2 * MT], F32, name=f"sto_{l}",
                                        tag=f"sto{l}", bufs=1,
                                        addr_space="Shared")
                    nc.sync.dma_start(out=st_in, in_=stats)
                    nc.gpsimd.collective_compute(
                        "AllReduce", ALU.add,
                        replica_groups=[list(range(NCORES))],
                        ins=[st_in.opt()], outs=[st_out.opt()])
                    nc.sync.dma_start(out=gsum, in_=st_out)
                else:
                    st_in = dramp.tile([128, 2 * MT], F32, name=f"sti_{l}",
                                       tag=f"sti{l}", bufs=1)
                    ag_out = dramp.tile([NCORES * 128, 2 * MT], F32,
                                        name=f"sto_{l}", tag=f"sto{l}",
                                        bufs=1, addr_space="Shared")
                    nc.sync.dma_start(out=st_in, in_=stats)
                    nc.gpsimd.collective_compute(
                        "AllGather", ALU.bypass,
                        replica_groups=[list(range(NCORES))],
                        ins=[st_in.opt()], outs=[ag_out.opt()])
                    gst = stp.tile([128, NCORES * 2 * MT], F32,
                                   name=f"gst_{l}", tag="gst")
                    gst_v = gst.rearrange("p (s t) -> p s t", s=NCORES)
                    ag_v = ag_out.rearrange("(s p) t -> p s t", s=NCORES)
                    half = NCORES // 2
                    nc.sync.dma_start(out=gst_v[:, 0:half],
                                      in_=ag_v[:, 0:half])
                    nc.gpsimd.dma_start(out=gst_v[:, half:NCORES],
                                        in_=ag_v[:, half:NCORES])

                # fill the collective window with unpool pre-work
                for _ in range(UNITS_PER_LAYER[l]):
                    if unit_pos < len(slab_units):
                        emit_slab_unit(*slab_units[unit_pos])
                        unit_pos += 1
                load_wf(l + 2)

                if not (DBG_NOAR or DBG_AR):
                    t1 = stp.tile([128, NCORES * MT], F32, name=f"t1_{l}",
                                  tag="t1")
                    nc.vector.tensor_add(t1, gst[:, 0:NCORES * MT],
                                         gst[:, NCORES * MT:2 * NCORES * MT])
                    t2 = stp.tile([128, NCORES * 2], F32, name=f"t2_{l}",
                                  tag="t2")
                    nc.vector.tensor_add(t2, t1[:, 0:NCORES * 2],
                                         t1[:, NCORES * 2:NCORES * MT])
                    nc.vector.tensor_add(gsum, t2[:, 0:2 * MT],
                                         t2[:, 2 * MT:NCORES * 2])

                mean = stp.tile([128, MT], F32, name=f"mean_{l}", tag="mean")
                nc.vector.tensor_scalar_mul(mean, gsum[:, 0:MT], INV_BC)
                msq = stp.tile([128, MT], F32, name=f"msq_{l}", tag="msq")
                nc.vector.tensor_scalar_mul(msq, gsum[:, MT:2 * MT], INV_BC)
                m2 = stp.tile([128, MT], F32, name=f"m2_{l}", tag="m2")
                nc.vector.tensor_mul(m2, mean, mean)
                var = stp.tile([128, MT], F32, name=f"var_{l}", tag="var")
                nc.vector.scalar_tensor_tensor(
                    out=var, in0=m2, scalar=-1.0, in1=msq,
                    op0=ALU.mult, op1=ALU.add)
                std = stp.tile([128, MT], F32, name=f"std_{l}", tag="std")
                nc.scalar.activation(out=std, in_=var, func=AF.Sqrt,
                                     bias=eps_sb)
                rstd = stp.tile([128, MT], F32, name=f"rstd_{l}", tag="rstd")
                nc.vector.reciprocal(out=rstd, in_=std)
                a_t = stp.tile([128, MT], F32, name=f"a_{l}", tag="a")
                nc.vector.tensor_mul(a_t, gb_sb[:, l * MT:(l + 1) * MT], rstd)
                ma = stp.tile([128, MT], F32, name=f"ma_{l}", tag="ma")
                nc.vector.tensor_mul(ma, mean, a_t)
                b_t = stp.tile([128, MT], F32, name=f"b_{l}", tag="b")
                nc.vector.scalar_tensor_tensor(
                    out=b_t, in0=ma, scalar=-1.0,
                    in1=bb_sb[:, l * MT:(l + 1) * MT],
                    op0=ALU.mult, op1=ALU.add)

                h_next = []
                for oi in range(MT):
                    tmp = sqp.tile([128, BL * C], F32, name=f"tmp_{l}_{oi}",
                                   tag="sq")
                    nc.vector.scalar_tensor_tensor(
                        out=tmp, in0=y_sb[oi], scalar=a_t[:, oi:oi + 1],
                        in1=h_cur[oi], op0=ALU.mult, op1=ALU.add)
                    hn = hp.tile([128, BL * C], BF16, name=f"h_{l + 1}_{oi}",
                                 tag="h")
                    with nc.allow_low_precision("h state in bf16"):
                        nc.scalar.activation(out=hn, in_=tmp, func=AF.Relu,
                                             bias=b_t[:, oi:oi + 1])
                    h_next.append(hn)
                h_cur = h_next
            h_fin = h_cur

            # ---------------- output phase ----------------
            h_fin8 = []
            for mi in range(MT):
                h8 = rzp.tile([128, BL * C], F8, name=f"h8_{mi}", tag="h8",
                              bufs=MT)
                with nc.allow_low_precision("h_fin in fp8 for output matmul"):
                    nc.vector.tensor_copy(out=h8, in_=h_fin[mi])
                h_fin8.append(h8)

            def emit_out_batch(b):
                dma_engines = [nc.sync, nc.scalar, nc.gpsimd, nc.sync]
                for j2 in range(NJ // 2):
                    ps_o = ps1.tile([128, 1024], F32, name=f"o_{b}_{j2}",
                                    tag="p1")
                    for mi in range(MT):
                        for k in range(2):
                            nj = 2 * j2 + k
                            nc.tensor.matmul(
                                ps_o[:, k * 512:(k + 1) * 512],
                                h_fin8[mi][:, b * C:(b + 1) * C],
                                eu_tiles[(b, mi)][:, nj * 512:(nj + 1) * 512],
                                start=(mi == 0), stop=(mi == MT - 1))
                    o_sb = osbp.tile([128, 1024], F32, name=f"os_{b}_{j2}",
                                     tag="os")
                    if DBG_NOPB:
                        for k in range(2):
                            rzb = rzbp.tile([128, 512], BF16,
                                            name=f"rzbo_{b}_{j2}_{k}",
                                            tag="rzbo")
                            bcast(rzb, rz_all[b][:, (2 * j2 + k) * 512:
                                                 (2 * j2 + k + 1) * 512],
                                  ps3, "p3z")
                            nc.vector.tensor_mul(
                                o_sb[:, k * 512:(k + 1) * 512],
                                ps_o[:, k * 512:(k + 1) * 512], rzb)
                    else:
                        rzb = rzbp.tile([128, 1024], BF16,
                                        name=f"rzbo_{b}_{j2}", tag="rzbo")
                        nc.gpsimd.partition_broadcast(
                            rzb, rz_all[b][:, 2 * j2 * 512:(2 * j2 + 2) * 512])
                        nc.vector.tensor_mul(o_sb, ps_o, rzb)
                    dma_engines[j2].dma_start(
                        out=out_d[b, :, 2 * j2 * 512:(2 * j2 + 2) * 512],
                        in_=o_sb)

            while unit_pos < len(slab_units):
                emit_slab_unit(*slab_units[unit_pos])
                unit_pos += 1
            for b in range(BL):
                emit_out_batch(b)


_CACHE = {}


def build():
    if "nc" in _CACHE:
        return _CACHE["nc"]
    nc = bacc.Bacc("TRN2", target_bir_lowering=False, debug=False,
                   num_devices=NCORES)
    x_d = nc.dram_tensor("x", [BL, C, N], BF16, kind="ExternalInput").ap()
    wpT_d = nc.dram_tensor("w_pool_t", [C, M], BF16, kind="ExternalInput").ap()
    wfT_d = nc.dram_tensor("wf_t", [L, M, M], BF16, kind="ExternalInput").ap()
    gamma_d = nc.dram_tensor("gamma", [L, M], F32, kind="ExternalInput").ap()
    beta_d = nc.dram_tensor("beta", [L, M], F32, kind="ExternalInput").ap()
    wuT_d = nc.dram_tensor("w_unpool_t", [C, M], BF16,
                           kind="ExternalInput").ap()
    bu_d = nc.dram_tensor("b_unpool", [M], F32, kind="ExternalInput").ap()
    ident_d = nc.dram_tensor("ident", [128, 128], BF16,
                             kind="ExternalInput").ap()
    ones_d = nc.dram_tensor("ones", [128, 128], BF16,
                            kind="ExternalInput").ap()
    out_d = nc.dram_tensor("out", [BL, C, N], F32, kind="ExternalOutput").ap()

    with tile.TileContext(nc) as tc:
        _kernel_body(nc, tc, x_d, wpT_d, wfT_d, gamma_d, beta_d, wuT_d, bu_d,
                     ident_d, ones_d, out_d)
    nc.compile()
    _CACHE["nc"] = nc
    return nc


def make_in_maps(x, W_pool, Wf, gamma, beta, W_unpool, b_unpool):
    bf16 = mybir.dt.np(BF16)
    xs = np.ascontiguousarray(
        np.asarray(x, dtype=np.float32)[..., 0]).astype(bf16)
    shards = xs.reshape(NCORES, BL, C, N)
    wpT = np.ascontiguousarray(np.asarray(W_pool, np.float32).T).astype(bf16)
    wfT = np.ascontiguousarray(
        np.asarray(Wf, np.float32).transpose(0, 2, 1)).astype(bf16)
    wuT = np.ascontiguousarray(np.asarray(W_unpool, np.float32).T).astype(bf16)
    common = {
        "w_pool_t": wpT, "wf_t": wfT,
        "gamma": np.ascontiguousarray(np.asarray(gamma, np.float32)),
        "beta": np.ascontiguousarray(np.asarray(beta, np.float32)),
        "w_unpool_t": wuT,
        "b_unpool": np.ascontiguousarray(np.asarray(b_unpool, np.float32)),
        "ident": np.eye(128, dtype=np.float32).astype(bf16),
        "ones": np.ones((128, 128), dtype=np.float32).astype(bf16),
    }
    return [{"x": np.ascontiguousarray(shards[i]), **common}
            for i in range(NCORES)]


LAST_RESULTS = None


def kernel(x, W_pool, Wf, gamma, beta, W_unpool, b_unpool, trace=False):
    global LAST_RESULTS
    from concourse.bass_utils import run_bass_kernel_spmd
    nc = build()
    in_maps = make_in_maps(x, W_pool, Wf, gamma, beta, W_unpool, b_unpool)
    res = run_bass_kernel_spmd(nc, in_maps, core_ids=list(range(NCORES)),
                               trace=trace)
    LAST_RESULTS = res
    out = np.concatenate([res.results[i]["out"] for i in range(NCORES)],
                         axis=0)
    return out.reshape(B, C, N, 1)


# revision 24
# speedup vs baseline: 2.9048x; 1.4593x over previous
"""Trainium2 Bass kernel for nn_Block_Order_Aware_Filtering_1_to_1 (v2).

Reference math (B=32, C=128, N=4096, M=512, L=6):
  xs = x[..., 0]                                            [B, C, N]
  Spool = softmax_n(W_pool @ xs)                            [B, M, N]
  h     = einsum('bcn,bmn->bmc', xs, Spool)                 [B, M, C]
  6x:  y = Wf[l] @ h ; BN over (B, C) ; h = relu(yn + h)
  ul  = W_unpool @ xs + b_unpool                            [B, M, N]
  Sun = softmax_m(ul)
  out = einsum('bcm,bmn->bcn', h^T, Sun)[..., None]

Sharding: data-parallel over B across 8 cores (BL=4 batch each); params
replicated; exact global-batch BN parity via one small AllGather of
(sum y, sum y^2) per layer + local 8-way reduction.

v2 layout/scheduling:
  - bf16 operands for all matmuls (PSUM accumulation stays f32); exp has
    no max-subtraction (|logit| <~ 4, exactly equal in exact arithmetic).
  - xsT via DMA xbar transpose (no PE transposes / DVE evacuations).
  - 1/Z row -> all partitions via gpsimd partition_broadcast (PE freed).
  - single long-lived pool set; PSUM budget 4+2+2 = 8 banks so the three
    phases can overlap.
  - unpool logits+exp+Z for batches 0-1 are interleaved into the 6 BN
    AllGather windows; batches 2-3 run inside the output phase.
"""

import os

import numpy as np

import concourse.bass as bass
import concourse.tile as tile
from concourse import bacc, mybir

F32 = mybir.dt.float32
BF16 = mybir.dt.bfloat16
F8 = mybir.dt.float8e4
AF = mybir.ActivationFunctionType
ALU = mybir.AluOpType

B, C, N, M, L = 32, 128, 4096, 512, 6
NCORES = 8
BL = B // NCORES          # 4 batch items per core
NT = N // 128             # 32 n-chunks of 128
MT = M // 128             # 4 m/o-chunks of 128
NJ = N // 512             # 8 n-tiles of 512
EPS = 1e-5
INV_BC = 1.0 / float(B * C)

DBG_LAYERS = int(os.environ.get("BASSK_LAYERS", str(L)))
DBG_NOAR = os.environ.get("BASSK_NOAR", "0") == "1"
DBG_REPS = int(os.environ.get("BASSK_REPS", "1"))
DBG_AR = os.environ.get("BASSK_AR", "0") == "1"        # AllReduce not AllGather
DBG_NOPB = os.environ.get("BASSK_NOPB", "0") == "1"    # PE bcast, no gpsimd pb
DBG_NODMAT = os.environ.get("BASSK_NODMAT", "0") == "1"  # PE transposes

# how many unpool-pre units (batch, nj-pair) to emit inside each filter
# layer's collective window; 16 units total (4 batches x 4 nj-pairs)
UNITS_PER_LAYER = [2, 2, 3, 3, 3, 3] if L == 6 else [16] + [0] * (L - 1)
if os.environ.get("BASSK_UNITS"):
    UNITS_PER_LAYER = [int(v) for v in os.environ["BASSK_UNITS"].split(",")]


def _kernel_body(nc, tc, x_d, wpT_d, wfT_d, gamma_d, beta_d, wuT_d, bu_d,
                 ident_d, ones_d, out_d):
    with (
        tc.tile_pool(name="const", bufs=1) as constp,
        tc.tile_pool(name="xs", bufs=BL) as xsp,
        tc.tile_pool(name="xsT", bufs=1) as xsTp,
        tc.tile_pool(name="E", bufs=4) as Ep,
        tc.tile_pool(name="h", bufs=8) as hp,
        tc.tile_pool(name="wf", bufs=8) as wfp,
        tc.tile_pool(name="ysb", bufs=4) as ysbp,
        tc.tile_pool(name="sq", bufs=2) as sqp,
        tc.tile_pool(name="st", bufs=2) as stp,
        tc.tile_pool(name="eu", bufs=16) as eup,
        tc.tile_pool(name="rz", bufs=1) as rzp,
        tc.tile_pool(name="rzb", bufs=3) as rzbp,
        tc.tile_pool(name="osb", bufs=2) as osbp,
        tc.tile_pool(name="ps1", bufs=2, space="PSUM") as ps1,   # 2x[128,1024]
        tc.tile_pool(name="ps2", bufs=2, space="PSUM") as ps2,   # 2x[128,512]
        tc.tile_pool(name="ps3", bufs=2, space="PSUM") as ps3,   # 2x 1 bank
        tc.tile_pool(name="dram", bufs=2, space="DRAM") as dramp,
    ):
        # ---------------- constants ----------------
        ident = constp.tile([128, 128], BF16)
        nc.sync.dma_start(out=ident, in_=ident_d)
        ones_col = constp.tile([128, 1], BF16)
        nc.sync.dma_start(out=ones_col, in_=ones_d[:, 0:1])
        ones8 = constp.tile([128, 1], F8)
        nc.vector.memset(ones8, 1.0)
        ones_row = constp.tile([1, 128], BF16)
        nc.sync.dma_start(out=ones_row, in_=ones_d[0:1, :])

        def bcast(out_sb, row_ap, psp, tag):
            """Broadcast [1, W] row_ap to all 128 partitions of out_sb."""
            if DBG_NOPB:
                w = row_ap.free_size()
                ps_rb = psp.tile([128, w], F32, tag=tag)
                nc.tensor.matmul(ps_rb, ones_row, row_ap, start=True,
                                 stop=True)
                nc.vector.tensor_copy(out=out_sb, in_=ps_rb)
            else:
                nc.gpsimd.partition_broadcast(out_sb, row_ap)
        wp_sb = constp.tile([C, M], BF16)
        nc.sync.dma_start(out=wp_sb, in_=wpT_d)
        wu_sb = constp.tile([C, M], BF16)
        nc.sync.dma_start(out=wu_sb, in_=wuT_d)
        gb_sb = constp.tile([128, L * MT], F32)
        nc.sync.dma_start(out=gb_sb.rearrange("p (l o) -> p l o", l=L),
                          in_=gamma_d.rearrange("l (o p) -> p l o", p=128))
        bb_sb = constp.tile([128, L * MT], F32)
        nc.sync.dma_start(out=bb_sb.rearrange("p (l o) -> p l o", l=L),
                          in_=beta_d.rearrange("l (o p) -> p l o", p=128))
        eps_sb = constp.tile([128, 1], F32)
        nc.vector.memset(eps_sb, EPS)
        bu_sb = constp.tile([128, MT], F32)
        nc.sync.dma_start(out=bu_sb, in_=bu_d.rearrange("(o p) -> p o", p=128))

        for _rep in range(DBG_REPS):
            # ---------------- pool phase ----------------
            xs_tiles = []
            h0 = [hp.tile([128, BL * C], BF16, name=f"h_0_{mi}", tag="h")
                  for mi in range(MT)]
            for b in range(BL):
                xs_sb = xsp.tile([C, N], BF16, name=f"xs_{b}", tag="xs")
                nc.sync.dma_start(out=xs_sb, in_=x_d[b])
                xs_tiles.append(xs_sb)
                xsT_sb = xsTp.tile([128, NT * C], BF16, name=f"xsT_{b}",
                                   tag="xsT")
                if DBG_NODMAT:
                    for ni in range(NT):
                        ps_t = ps3.tile([128, 128], BF16,
                                        name=f"xt_{b}_{ni}", tag="p3z")
                        nc.tensor.transpose(
                            ps_t, xs_sb[:, ni * 128:(ni + 1) * 128], ident)
                        nc.vector.tensor_copy(
                            out=xsT_sb[:, ni * C:(ni + 1) * C], in_=ps_t)
                else:
                    nc.sync.dma_start_transpose(
                        out=xsT_sb.rearrange("p (t c) -> p t c", t=NT),
                        in_=x_d[b])

                ps_hT = ps2.tile([128, M], F32, name=f"hT_{b}", tag="p2")
                ps_z = ps2.tile([1, M], F32, name=f"z_{b}", tag="p2")

                def consume(g, b=b, xsT_sb=xsT_sb, ps_hT=ps_hT, ps_z=ps_z,
                            E_sb=None):
                    for k in range(2):
                        ni = 2 * g + k
                        nc.tensor.matmul(
                            ps_hT, xsT_sb[:, ni * C:(ni + 1) * C],
                            E_sb[ni][:, (ni % 2) * M:(ni % 2 + 1) * M],
                            start=(ni == 0), stop=(ni == NT - 1))
                        nc.tensor.matmul(
                            ps_z[0:1, :], ones_col,
                            E_sb[ni][:, (ni % 2) * M:(ni % 2 + 1) * M],
                            start=(ni == 0), stop=(ni == NT - 1))

                E_of_ni = {}
                for g in range(NT // 2):
                    ps_log = ps1.tile([128, 2 * M], F32, name=f"log_{b}_{g}",
                                      tag="p1")
                    for k in range(2):
                        ni = 2 * g + k
                        nc.tensor.matmul(ps_log[:, k * M:(k + 1) * M],
                                         xs_sb[:, ni * 128:(ni + 1) * 128],
                                         wp_sb, start=True, stop=True)
                    e_t = Ep.tile([128, 2 * M], BF16, name=f"E_{b}_{g}",
                                  tag="E")
                    with nc.allow_low_precision("softmax weights in bf16"):
                        nc.scalar.activation(out=e_t, in_=ps_log, func=AF.Exp)
                    E_of_ni[2 * g] = e_t
                    E_of_ni[2 * g + 1] = e_t
                    if g >= 1:
                        consume(g - 1, E_sb=E_of_ni)
                consume(NT // 2 - 1, E_sb=E_of_ni)

                rz = rzp.tile([1, M], BF16, name=f"rz_{b}", tag="rzpool",
                              bufs=2)
                with nc.allow_low_precision("pool 1/Z in bf16"):
                    nc.vector.reciprocal(out=rz, in_=ps_z[0:1, :])
                rzb = rzbp.tile([128, M], BF16, name=f"rzb_{b}", tag="rzbp")
                bcast(rzb, rz, ps3, "p3z")
                hTs = rzp.tile([128, M], BF16, name=f"hTs_{b}", tag="hTs",
                               bufs=2)
                with nc.allow_low_precision("pooled h in bf16"):
                    nc.vector.tensor_mul(hTs, ps_hT, rzb)
                for mi in range(MT):
                    ps_h = ps2.tile([128, 128], BF16, name=f"h_{b}_{mi}",
                                    tag="p2")
                    nc.tensor.transpose(ps_h, hTs[:, mi * 128:(mi + 1) * 128],
                                        ident)
                    nc.vector.tensor_copy(out=h0[mi][:, b * C:(b + 1) * C],
                                          in_=ps_h)

            # -------- filter phase with interleaved unpool-pre slabs -------
            eu_tiles = {}    # (b, mi) -> [128, N] fp8e4m3
            rz_all = {}      # b -> [1, NJ*512] bf16
            slab_units = [(b, j2) for b in range(BL) for j2 in range(NJ // 2)]
            unit_pos = 0

            def emit_slab_unit(b, j2):
                xs_sb = xs_tiles[b]
                if b not in rz_all:
                    rz_all[b] = rzp.tile([1, NJ * 512], BF16,
                                         name=f"rzu_{b}", tag=f"rzu{b}",
                                         bufs=1)
                eus = []
                for mi in range(MT):
                    if (b, mi) not in eu_tiles:
                        eu_tiles[(b, mi)] = eup.tile(
                            [128, N], F8, name=f"eu_{b}_{mi}", tag="eu")
                    eu = eu_tiles[(b, mi)]
                    ps_ul = ps1.tile([128, 1024], F32,
                                     name=f"ul_{b}_{j2}_{mi}", tag="p1")
                    for k in range(2):
                        nj = 2 * j2 + k
                        nc.tensor.matmul(
                            ps_ul[:, k * 512:(k + 1) * 512],
                            wu_sb[:, mi * 128:(mi + 1) * 128],
                            xs_sb[:, nj * 512:(nj + 1) * 512],
                            start=True, stop=True)
                    with nc.allow_low_precision("softmax weights in bf16"):
                        nc.scalar.activation(
                            out=eu[:, 2 * j2 * 512:(2 * j2 + 2) * 512],
                            in_=ps_ul, func=AF.Exp,
                            bias=bu_sb[:, mi:mi + 1])
                    eus.append(eu)
                for k in range(2):
                    nj = 2 * j2 + k
                    ps_zu = ps3.tile([1, 512], F32, name=f"zu_{b}_{nj}",
                                     tag="p3z")
                    for mi in range(MT):
                        nc.tensor.matmul(
                            ps_zu[0:1, :], ones8,
                            eus[mi][:, nj * 512:(nj + 1) * 512],
                            start=(mi == 0), stop=(mi == MT - 1))
                    with nc.allow_low_precision("1/Z in bf16"):
                        nc.vector.reciprocal(
                            out=rz_all[b][:, nj * 512:(nj + 1) * 512],
                            in_=ps_zu[0:1, :])

            h_cur = h0
            wf_sb = {}

            def load_wf(l):
                if l >= DBG_LAYERS:
                    return
                tiles = []
                for mi in range(MT):
                    w = wfp.tile([128, M], BF16, name=f"wf_{l}_{mi}",
                                 tag="wf")
                    nc.gpsimd.dma_start(
                        out=w, in_=wfT_d[l, mi * 128:(mi + 1) * 128, :])
                    tiles.append(w)
                wf_sb[l] = tiles

            load_wf(0)
            load_wf(1)

            for l in range(DBG_LAYERS):
                stats = stp.tile([128, 2 * MT], F32, name=f"st_{l}",
                                 tag="st")
                y_sb = []
                for oi in range(MT):
                    ps_y = ps2.tile([128, BL * C], F32, name=f"y_{l}_{oi}",
                                    tag="p2")
                    for mi in range(MT):
                        nc.tensor.matmul(
                            ps_y, wf_sb[l][mi][:, oi * 128:(oi + 1) * 128],
                            h_cur[mi], start=(mi == 0), stop=(mi == MT - 1))
                    y = ysbp.tile([128, BL * C], F32, name=f"ysb_{l}_{oi}",
                                  tag="ysb")
                    nc.scalar.activation(out=y, in_=ps_y, func=AF.Copy,
                                         accum_out=stats[:, oi:oi + 1])
                    sq = sqp.tile([128, BL * C], F32, name=f"sq_{l}_{oi}",
                                  tag="sq")
                    nc.vector.scalar_tensor_tensor(
                        out=sq, in0=y, scalar=1.0, in1=y,
                        op0=ALU.mult, op1=ALU.mult,
                        accum_out=stats[:, MT + oi:MT + oi + 1])
                    y_sb.append(y)

                # exchange partial sums -> exact global-batch BN stats
                gsum = stp.tile([128, 2 * MT], F32, name=f"gsum_{l}",
                                tag="gsum")
                if DBG_NOAR:
                    nc.vector.tensor_scalar_mul(gsum, stats, float(NCORES))
                elif DBG_AR:
                    st_in = dramp.tile([128, 2 * MT], F32, name=f"sti_{l}",
                                       tag=f"sti{l}", bufs=1)
                    st_out = dramp.tile([128, 2 * MT], F32, name=f"sto_{l}",
                                        tag=f"sto{l}", bufs=1,
                                        addr_space="Shared")
                    nc.sync.dma_start(out=st_in, in_=stats)
                    nc.gpsimd.collective_compute(
                        "AllReduce", ALU.add,
                        replica_groups=[list(range(NCORES))],
                        ins=[st_in.opt()], outs=[st_out.opt()])
                    nc.sync.dma_start(out=gsum, in_=st_out)
                else:
                    st_in = dramp.tile([128, 2 * MT], F32, name=f"sti_{l}",
                                       tag=f"sti{l}", bufs=1)
                    ag_out = dramp.tile([NCORES * 128, 2 * MT], F32,
                                        name=f"sto_{l}", tag=f"sto{l}",
                                        bufs=1, addr_space="Shared")
                    nc.sync.dma_start(out=st_in, in_=stats)
                    nc.gpsimd.collective_compute(
                        "AllGather", ALU.bypass,
                        replica_groups=[list(range(NCORES))],
                        ins=[st_in.opt()], outs=[ag_out.opt()])
                    gst = stp.tile([128, NCORES * 2 * MT], F32,
                                   name=f"gst_{l}", tag="gst")
                    gst_v = gst.rearrange("p (s t) -> p s t", s=NCORES)
                    ag_v = ag_out.rearrange("(s p) t -> p s t", s=NCORES)
                    half = NCORES // 2
                    nc.sync.dma_start(out=gst_v[:, 0:half],
                                      in_=ag_v[:, 0:half])
                    nc.gpsimd.dma_start(out=gst_v[:, half:NCORES],
                                        in_=ag_v[:, half:NCORES])

                # fill the collective window with unpool pre-work
                for _ in range(UNITS_PER_LAYER[l]):
                    if unit_pos < len(slab_units):
                        emit_slab_unit(*slab_units[unit_pos])
                        unit_pos += 1
                load_wf(l + 2)

                if not (DBG_NOAR or DBG_AR):
                    t1 = stp.tile([128, NCORES * MT], F32, name=f"t1_{l}",
                                  tag="t1")
                    nc.vector.tensor_add(t1, gst[:, 0:NCORES * MT],
                                         gst[:, NCORES * MT:2 * NCORES * MT])
                    t2 = stp.tile([128, NCORES * 2], F32, name=f"t2_{l}",
                                  tag="t2")
                    nc.vector.tensor_add(t2, t1[:, 0:NCORES * 2],
                                         t1[:, NCORES * 2:NCORES * MT])
                    nc.vector.tensor_add(gsum, t2[:, 0:2 * MT],
                                         t2[:, 2 * MT:NCORES * 2])

                mean = stp.tile([128, MT], F32, name=f"mean_{l}", tag="mean")
                nc.vector.tensor_scalar_mul(mean, gsum[:, 0:MT], INV_BC)
                msq = stp.tile([128, MT], F32, name=f"msq_{l}", tag="msq")
                nc.vector.tensor_scalar_mul(msq, gsum[:, MT:2 * MT], INV_BC)
                m2 = stp.tile([128, MT], F32, name=f"m2_{l}", tag="m2")
                nc.vector.tensor_mul(m2, mean, mean)
                var = stp.tile([128, MT], F32, name=f"var_{l}", tag="var")
                nc.vector.scalar_tensor_tensor(
                    out=var, in0=m2, scalar=-1.0, in1=msq,
                    op0=ALU.mult, op1=ALU.add)
                std = stp.tile([128, MT], F32, name=f"std_{l}", tag="std")
                nc.scalar.activation(out=std, in_=var, func=AF.Sqrt,
                                     bias=eps_sb)
                rstd = stp.tile([128, MT], F32, name=f"rstd_{l}", tag="rstd")
                nc.vector.reciprocal(out=rstd, in_=std)
                a_t = stp.tile([128, MT], F32, name=f"a_{l}", tag="a")
                nc.vector.tensor_mul(a_t, gb_sb[:, l * MT:(l + 1) * MT], rstd)
                ma = stp.tile([128, MT], F32, name=f"ma_{l}", tag="ma")
                nc.vector.tensor_mul(ma, mean, a_t)
                b_t = stp.tile([128, MT], F32, name=f"b_{l}", tag="b")
                nc.vector.scalar_tensor_tensor(
                    out=b_t, in0=ma, scalar=-1.0,
                    in1=bb_sb[:, l * MT:(l + 1) * MT],
                    op0=ALU.mult, op1=ALU.add)

                h_next = []
                for oi in range(MT):
                    tmp = sqp.tile([128, BL * C], F32, name=f"tmp_{l}_{oi}",
                                   tag="sq")
                    nc.vector.scalar_tensor_tensor(
                        out=tmp, in0=y_sb[oi], scalar=a_t[:, oi:oi + 1],
                        in1=h_cur[oi], op0=ALU.mult, op1=ALU.add)
                    hn = hp.tile([128, BL * C], BF16, name=f"h_{l + 1}_{oi}",
                                 tag="h")
                    with nc.allow_low_precision("h state in bf16"):
                        nc.scalar.activation(out=hn, in_=tmp, func=AF.Relu,
                                             bias=b_t[:, oi:oi + 1])
                    h_next.append(hn)
                h_cur = h_next
            h_fin = h_cur

            # ---------------- output phase ----------------
            h_fin8 = []
            for mi in range(MT):
                h8 = rzp.tile([128, BL * C], F8, name=f"h8_{mi}", tag="h8",
                              bufs=MT)
                with nc.allow_low_precision("h_fin in fp8 for output matmul"):
                    nc.vector.tensor_copy(out=h8, in_=h_fin[mi])
                h_fin8.append(h8)

            def emit_out_batch(b):
                dma_engines = [nc.sync, nc.scalar, nc.gpsimd, nc.sync]
                for j2 in range(NJ // 2):
                    ps_o = ps1.tile([128, 1024], F32, name=f"o_{b}_{j2}",
                                    tag="p1")
                    for mi in range(MT):
                        for k in range(2):
                            nj = 2 * j2 + k
                            nc.tensor.matmul(
                                ps_o[:, k * 512:(k + 1) * 512],
                                h_fin8[mi][:, b * C:(b + 1) * C],
                                eu_tiles[(b, mi)][:, nj * 512:(nj + 1) * 512],
                                start=(mi == 0), stop=(mi == MT - 1))
                    o_sb = osbp.tile([128, 1024], F32, name=f"os_{b}_{j2}",
                                     tag="os")
                    if DBG_NOPB:
                        for k in range(2):
                            rzb = rzbp.tile([128, 512], BF16,
                                            name=f"rzbo_{b}_{j2}_{k}",
                                            tag="rzbo")
                            bcast(rzb, rz_all[b][:, (2 * j2 + k) * 512:
                                                 (2 * j2 + k + 1) * 512],
                                  ps3, "p3z")
                            nc.vector.tensor_mul(
                                o_sb[:, k * 512:(k + 1) * 512],
                                ps_o[:, k * 512:(k + 1) * 512], rzb)
                    else:
                        rzb = rzbp.tile([128, 1024], BF16,
                                        name=f"rzbo_{b}_{j2}", tag="rzbo")
                        nc.gpsimd.partition_broadcast(
                            rzb, rz_all[b][:, 2 * j2 * 512:(2 * j2 + 2) * 512])
                        nc.vector.tensor_mul(o_sb, ps_o, rzb)
                    dma_engines[j2].dma_start(
                        out=out_d[b, :, 2 * j2 * 512:(2 * j2 + 2) * 512],
                        in_=o_sb)

            while unit_pos < len(slab_units):
                emit_slab_unit(*slab_units[unit_pos])
                unit_pos += 1
            for b in range(BL):
                emit_out_batch(b)


_CACHE = {}


def build():
    if "nc" in _CACHE:
        return _CACHE["nc"]
    nc = bacc.Bacc("TRN2", target_bir_lowering=False, debug=False,
                   num_devices=NCORES)
    x_d = nc.dram_tensor("x", [BL, C, N], BF16, kind="ExternalInput").ap()
    wpT_d = nc.dram_tensor("w_pool_t", [C, M], BF16, kind="ExternalInput").ap()
    wfT_d = nc.dram_tensor("wf_t", [L, M, M], BF16, kind="ExternalInput").ap()
    gamma_d = nc.dram_tensor("gamma", [L, M], F32, kind="ExternalInput").ap()
    beta_d = nc.dram_tensor("beta", [L, M], F32, kind="ExternalInput").ap()
    wuT_d = nc.dram_tensor("w_unpool_t", [C, M], BF16,
                           kind="ExternalInput").ap()
    bu_d = nc.dram_tensor("b_unpool", [M], F32, kind="ExternalInput").ap()
    ident_d = nc.dram_tensor("ident", [128, 128], BF16,
                             kind="ExternalInput").ap()
    ones_d = nc.dram_tensor("ones", [128, 128], BF16,
                            kind="ExternalInput").ap()
    out_d = nc.dram_tensor("out", [BL, C, N], F32, kind="ExternalOutput").ap()

    with tile.TileContext(nc) as tc:
        _kernel_body(nc, tc, x_d, wpT_d, wfT_d, gamma_d, beta_d, wuT_d, bu_d,
                     ident_d, ones_d, out_d)
    nc.compile()
    _CACHE["nc"] = nc
    return nc


def make_in_maps(x, W_pool, Wf, gamma, beta, W_unpool, b_unpool):
    bf16 = mybir.dt.np(BF16)
    xs = np.ascontiguousarray(
        np.asarray(x, dtype=np.float32)[..., 0]).astype(bf16)
    shards = xs.reshape(NCORES, BL, C, N)
    wpT = np.ascontiguousarray(np.asarray(W_pool, np.float32).T).astype(bf16)
    wfT = np.ascontiguousarray(
        np.asarray(Wf, np.float32).transpose(0, 2, 1)).astype(bf16)
    wuT = np.ascontiguousarray(np.asarray(W_unpool, np.float32).T).astype(bf16)
    common = {
        "w_pool_t": wpT, "wf_t": wfT,
        "gamma": np.ascontiguousarray(np.asarray(gamma, np.float32)),
        "beta": np.ascontiguousarray(np.asarray(beta, np.float32)),
        "w_unpool_t": wuT,
        "b_unpool": np.ascontiguousarray(np.asarray(b_unpool, np.float32)),
        "ident": np.eye(128, dtype=np.float32).astype(bf16),
        "ones": np.ones((128, 128), dtype=np.float32).astype(bf16),
    }
    return [{"x": np.ascontiguousarray(shards[i]), **common}
            for i in range(NCORES)]


LAST_RESULTS = None


def kernel(x, W_pool, Wf, gamma, beta, W_unpool, b_unpool, trace=False):
    global LAST_RESULTS
    from concourse.bass_utils import run_bass_kernel_spmd
    nc = build()
    in_maps = make_in_maps(x, W_pool, Wf, gamma, beta, W_unpool, b_unpool)
    res = run_bass_kernel_spmd(nc, in_maps, core_ids=list(range(NCORES)),
                               trace=trace)
    LAST_RESULTS = res
    out = np.concatenate([res.results[i]["out"] for i in range(NCORES)],
                         axis=0)
    return out.reshape(B, C, N, 1)
